# revision 20
# baseline (speedup 1.0000x reference)
"""Mixture-of-Experts Trainium2 kernel (8-core SPMD, token-sharded).

Reference computation: coarse top-K expert selection from the gate applied
to the global token sum, then dense K-expert FFN over all tokens with
per-token softmax gating over the K selected experts.

Strategy (V5, all bf16 — fp8/DoubleRow was measured 2x faster on PE but
every scheme with a single-e4m3 tensor exceeds the 2e-2 error budget, and
split schemes cost >= bf16):
  * Host: coarse routing (top-K), gather selected experts' params, AND the
    per-token gating softmax gw[T,K] computed exactly in fp32 (no gating
    work on device). Pre-cast params/x to bf16, x transposed feature-major.
    Token-shard across 8 cores, replicate selected-expert weights.
  * Device (per core): per expert k:
      L1: h[F,T] = gelu(W1_k.T @ xT + b1)   (tcc -> fc -> dc loops)
      L2: eo[T,D] = h_tile.T @ W2_k; acc (+)= eo * gw[:,k]  (DVE)
    Startup latency: each DMA trigger costs ~565ns on the SP sequencer, so
    weights/x live in single wide SBUF tiles loaded by a handful of big
    DMAs; expert-0 w1 and x stream in four 512-col chunks, interleaved, so
    the first L1 sweep starts after ~1MB. All DMA on the sync queue
    (Activation-engine triggers stall the gelu stream).
  * Output is written bf16 (halves out-DMA; ~2e-3 extra rounding, well
    inside the 2e-2 budget) and upcast on host.
"""

import numpy as np
import ml_dtypes
from contextlib import ExitStack

import bass_rust as _bass_rust
import concourse.bass as bass
import concourse.mybir as mybir
import concourse.tile as tile
from concourse.bass_utils import run_bass_kernel_spmd

BF16 = mybir.dt.bfloat16
FP8 = mybir.dt.float8e4
F32 = mybir.dt.float32
DR = mybir.MatmulPerfMode.DoubleRow
N_CORES = 8
P = 128


# ---------------------------------------------------------------------------
# Workaround for walrus "Too many sync wait commands": this walrus build
# accepts at most one semaphore wait in a single instruction's sync_info,
# but Tile's scheduler (and its kernel-tail drain) can attach several.
# Post-pass: move excess waits onto standalone EventSemaphore instructions
# inserted immediately before the offender on the same engine.
# ---------------------------------------------------------------------------
_split_ctr = [0]


def _split_multi_waits(nc):
    for f in nc.m.functions:
        for blk in f.blocks:
            insts = blk.instructions
            i = 0
            while i < len(insts):
                inst = insts[i]
                si = getattr(inst, "sync_info", None)
                waits = list(si.on_wait) if si is not None and si.on_wait else []
                if len(waits) > 1:
                    si.on_wait = waits[-1:]
                    for w in waits[:-1]:
                        _split_ctr[0] += 1
                        ev = mybir.InstEventSemaphore(
                            name=f"I-wsplit-{_split_ctr[0]}", ins=[], outs=[]
                        )
                        ev.engine = inst.engine
                        ev.sync_info = _bass_rust.SyncInfo(
                            on_wait=[w], on_update=[]
                        )
                        insts.insert(i, ev)
                        i += 1
                i += 1


# ---------------------------------------------------------------------------
# Device kernel
# ---------------------------------------------------------------------------
def build_moe_kernel(K: int, T: int, D: int, DF: int):
    """Per-core kernel: T tokens, D model dim, DF ffn dim, K selected experts."""
    assert T % 512 == 0 and D % P == 0 and DF % P == 0
    TT = T // P       # 128-token tiles
    TC = T // 512     # 512-token chunks
    DC = D // P       # D chunks of 128
    FC = DF // P      # F chunks of 128

    nc = bass.Bass("TRN2", target_bir_lowering=False)

    FB = FC - 2       # bf16 feature blocks in L2; last 2 go fp8 DoubleRow

    xT = nc.declare_dram_parameter("xT", [D, T], BF16, isOutput=False)
    w1s = nc.declare_dram_parameter("w1s", [K, D, DF], BF16, isOutput=False)
    w2s = nc.declare_dram_parameter("w2s", [K, DF, D], BF16, isOutput=False)
    w28 = nc.declare_dram_parameter("w28", [K, P, 2, D], FP8, isOutput=False)
    gwp = nc.declare_dram_parameter("gwp", [P, TT * K], F32, isOutput=False)
    b1p = nc.declare_dram_parameter("b1p", [P, K * FC], F32, isOutput=False)
    out = nc.declare_dram_parameter("out", [T, D], BF16, isOutput=True)

    mult = mybir.AluOpType.mult
    add = mybir.AluOpType.add
    gelu_fn = mybir.ActivationFunctionType.Gelu_apprx_tanh

    with tile.TileContext(nc) as tc:
        with ExitStack() as ctx:
            persist = ctx.enter_context(tc.tile_pool(name="persist", bufs=1))
            w1pool = ctx.enter_context(tc.tile_pool(name="w1p", bufs=2))
            w2pool = ctx.enter_context(tc.tile_pool(name="w2p", bufs=2))
            hp = ctx.enter_context(tc.tile_pool(name="hp", bufs=FC))
            h8p = ctx.enter_context(tc.tile_pool(name="h8p", bufs=2))
            w28p = ctx.enter_context(tc.tile_pool(name="w28p", bufs=2))
            obp = ctx.enter_context(tc.tile_pool(name="obp", bufs=3))
            psA = ctx.enter_context(tc.tile_pool(name="psA", bufs=6, space="PSUM"))
            psB = ctx.enter_context(tc.tile_pool(name="psB", bufs=2, space="PSUM"))

            # ---- startup-latency-critical loads (few, big, ordered) ----
            # Triggers cost ~565ns each on SP, so order by first-use time:
            # b1 (first gelu), w1[0] chunk 0 + x chunk 0 (first L1 sweep),
            # then the rest of w1[0] (the tcc=0 sweep spans all of DF),
            # then the remaining x chunks, then gw (first used by L2).
            b1_sb = persist.tile([P, K * FC], F32, tag="b1", name="b1_sb")
            nc.sync.dma_start(b1_sb[:], b1p[:])

            # w1 per expert: one [P, DC, DF] tile; x: one [P, DC, T] tile.
            w1t0 = w1pool.tile([P, DC, DF], BF16, tag="w1", name="w1_0")
            xc = persist.tile([P, DC, T], BF16, tag="xc", name="xc")
            w1r = w1s[0].rearrange("(dc p) f -> p dc f", dc=DC)
            xr = xT.rearrange("(dc p) t -> p dc t", dc=DC)

            def chunk(c):
                return slice(c * 512, (c + 1) * 512)

            nc.sync.dma_start(w1t0[:, :, chunk(0)], w1r[:, :, chunk(0)])
            nc.sync.dma_start(xc[:, :, chunk(0)], xr[:, :, chunk(0)])
            for c in range(1, TC):
                nc.sync.dma_start(w1t0[:, :, chunk(c)], w1r[:, :, chunk(c)])
            for c in range(1, TC):
                nc.sync.dma_start(xc[:, :, chunk(c)], xr[:, :, chunk(c)])
            gw_sb = persist.tile([P, TT * K], F32, tag="gw", name="gw_sb")
            nc.sync.dma_start(gw_sb[:], gwp[:])

            acc = [
                persist.tile([P, D], F32, tag=f"acc{t}", name=f"acc{t}")
                for t in range(TT)
            ]

            # ---- experts ----
            for k in range(K):
                if k == 0:
                    w1t = w1t0
                else:
                    w1t = w1pool.tile([P, DC, DF], BF16, tag="w1", name=f"w1_{k}")
                    nc.sync.dma_start(
                        w1t[:], w1s[k].rearrange("(dc p) f -> p dc f", dc=DC)
                    )
                w2t = w2pool.tile([P, FB, D], BF16, tag="w2", name=f"w2_{k}")
                nc.sync.dma_start(
                    w2t[:], w2s[k].rearrange("(fc p) d -> p fc d", fc=FC)[:, 0:FB, :]
                )
                w28t = w28p.tile([P, 2, D], FP8, tag="w28", name=f"w28_{k}")
                nc.sync.dma_start(w28t[:], w28[k])

                # L1: h[F,T] = gelu(W1.T @ x + b1), feature-major.
                # tcc -> fc -> dc: the first sweep only needs x chunk 0 and
                # the first w1 column-chunks as they stream in.
                ht = [
                    hp.tile([P, T], BF16, tag="h", name=f"h_{k}_{fc}")
                    for fc in range(FB)
                ]
                # fp8 h for the last 2 feature blocks, packed [P, TT, 2, P] so
                # each [P, 2, P] DoubleRow stationary slice is contiguous.
                h8 = h8p.tile([P, TT, 2, P], FP8, tag="h8", name=f"h8_{k}")
                for tcc in range(TC):
                    for fc in range(FC):
                        ph = psA.tile([P, 512], F32, tag="ph", name=f"ph_{k}_{fc}_{tcc}")
                        for dc in range(DC):
                            nc.tensor.matmul(
                                ph[:],
                                w1t[:, dc, fc * P:(fc + 1) * P],
                                xc[:, dc, tcc * 512:(tcc + 1) * 512],
                                start=(dc == 0),
                                stop=(dc == DC - 1),
                            )
                        bias = b1_sb[:, k * FC + fc:k * FC + fc + 1]
                        if fc < FB:
                            nc.scalar.activation(
                                ht[fc][:, tcc * 512:(tcc + 1) * 512], ph[:],
                                gelu_fn, bias=bias,
                            )
                        else:
                            nc.scalar.activation(
                                h8[:, 4 * tcc:4 * tcc + 4, fc - FB, :],
                                ph[:].rearrange("p (a b) -> p a b", a=4),
                                gelu_fn, bias=bias,
                            )

                # L2: eo[T,D] = h.T @ W2 ; acc (+)= eo * gw[:,k]
                for tt in range(TT):
                    po = psB.tile([P, 512], F32, tag="po", name=f"po_{k}_{tt}")
                    for fc in range(FB):
                        nc.tensor.matmul(
                            po[:, 0:D],
                            ht[fc][:, tt * P:(tt + 1) * P],
                            w2t[:, fc, :],
                            start=(fc == 0),
                            stop=False,
                        )
                    nc.tensor.matmul(
                        po[:, 0:D], h8[:, tt, :, :], w28t[:],
                        start=False, stop=True, perf_mode=DR,
                    )
                    gcol = k * TT + tt
                    if k == 0:
                        nc.vector.tensor_scalar_mul(
                            acc[tt][:], po[:, 0:D], gw_sb[:, gcol:gcol + 1]
                        )
                    elif k < K - 1:
                        nc.vector.scalar_tensor_tensor(
                            acc[tt][:], po[:, 0:D], gw_sb[:, gcol:gcol + 1],
                            acc[tt][:], op0=mult, op1=add,
                        )
                    else:
                        ob = obp.tile([P, D], BF16, tag="ob", name=f"ob_{tt}")
                        nc.vector.scalar_tensor_tensor(
                            ob[:], po[:, 0:D], gw_sb[:, gcol:gcol + 1],
                            acc[tt][:], op0=mult, op1=add,
                        )
                        nc.sync.dma_start(out[tt * P:(tt + 1) * P, :], ob[:])

    _split_multi_waits(nc)
    return nc


# ---------------------------------------------------------------------------
# Host wrapper
# ---------------------------------------------------------------------------
_NC_CACHE: dict = {}


def _get_nc(K: int, T: int, D: int, DF: int):
    key = (K, T, D, DF)
    if key not in _NC_CACHE:
        _NC_CACHE[key] = build_moe_kernel(K, T, D, DF)
    return _NC_CACHE[key]


def _softmax(x, axis=-1):
    m = np.max(x, axis=axis, keepdims=True)
    e = np.exp(x - m)
    return e / np.sum(e, axis=axis, keepdims=True)


def run(inputs: dict, trace: bool = False, tmpdir: str | None = None):
    x = np.asarray(inputs["x"], dtype=np.float32)
    gate_w = np.asarray(inputs["gate_w"], dtype=np.float32)
    gate_b = np.asarray(inputs["gate_b"], dtype=np.float32)
    w1 = np.asarray(inputs["w1"], dtype=np.float32)
    b1 = np.asarray(inputs["b1"], dtype=np.float32)
    w2 = np.asarray(inputs["w2"], dtype=np.float32)
    b2 = np.asarray(inputs["b2"], dtype=np.float32)
    K = int(inputs["num_available"])

    B, S, D = x.shape
    DF = w1.shape[2]
    Ttot = B * S
    T = Ttot // N_CORES
    TT = T // P
    FC = DF // P

    # Coarse routing on host (tiny): gate applied to the global token sum.
    ksum = x.sum(axis=(0, 1))
    coarse = gate_w @ ksum + gate_b
    idx = np.argsort(-coarse, kind="stable")[:K]

    gws = gate_w[idx]                      # [K, D]
    gbs = gate_b[idx]                      # [K]
    w1s = np.ascontiguousarray(w1[idx]).astype(ml_dtypes.bfloat16)   # [K,D,DF]
    b1s = np.ascontiguousarray(b1[idx], dtype=np.float32)            # [K,DF]
    w2s = np.ascontiguousarray(w2[idx]).astype(ml_dtypes.bfloat16)   # [K,DF,D]
    b2s = np.ascontiguousarray(b2[idx], dtype=np.float32)            # [K,D]
    # Last two DF blocks of w2 as natural-scale fp8 for the L2 DoubleRow
    # matmul: w28[k, p, s, d] = w2[idx[k], (FC-2+s)*128 + p, d]
    w28 = np.ascontiguousarray(
        w2[idx][:, (FC - 2) * P:, :].reshape(K, 2, P, D).transpose(0, 2, 1, 3)
    ).astype(ml_dtypes.float8_e4m3)

    # b1 packed [P, K*FC]: column k*FC+fc = b1[k, fc*128:(fc+1)*128]
    b1p = np.ascontiguousarray(
        b1s.reshape(K, FC, P).transpose(2, 0, 1).reshape(P, K * FC),
        dtype=np.float32,
    )

    xf = x.reshape(Ttot, D)
    xT_bf = np.ascontiguousarray(xf.T).astype(ml_dtypes.bfloat16)    # [D, Ttot]

    # Exact per-token gating softmax on host.
    logits = xf @ gws.T + gbs[None, :]     # [Ttot, K]
    gw = _softmax(logits, axis=1).astype(np.float32)
    gwp_all = gw.reshape(N_CORES, TT, P, K)

    nc = _get_nc(K, T, D, DF)
    in_maps = []
    for c in range(N_CORES):
        gwp = np.ascontiguousarray(
            gwp_all[c].transpose(1, 2, 0).reshape(P, K * TT), dtype=np.float32
        )
        in_maps.append({
            "xT": np.ascontiguousarray(xT_bf[:, c * T:(c + 1) * T]),
            "w1s": w1s,
            "w2s": w2s,
            "w28": w28,
            "gwp": gwp,
            "b1p": b1p,
        })

    res = run_bass_kernel_spmd(
        nc, in_maps, list(range(N_CORES)), trace=trace, tmpdir=tmpdir
    )
    outp = np.concatenate(
        [np.asarray(res.results[c]["out"]) for c in range(N_CORES)], axis=0
    ).astype(np.float32).reshape(B, S, D)

    # b2 contribution (zero in this problem's inputs; exact host-side fallback)
    if np.any(b2s):
        outp = outp + (gw @ b2s).reshape(B, S, D)

    return outp, res


def kernel(**inputs) -> np.ndarray:
    outp, _ = run(inputs, trace=False)
    return outp
